# revision 1
# baseline (speedup 1.0000x reference)
"""CTC loss (warp-ctc semantics, size_average=True) on 8 Trainium2 NeuronCores.

Strategy (data-parallel over batch, 4 samples per core):
- Z[t,b] = sum_v exp(acts[t,b,v]) streamed as [128,8000] tiles; exp + free-dim
  sum fused in one ScalarE activation (accum_out). Host does log Z in float64.
- The alpha recursion runs in the LINEAR domain on unnormalized p~ = exp(acts
  at extended labels). States split into blank block (101) / label block (100),
  laid out [state-partition, (t,b)-free]. Using blank' = blank + shift(label)
  and label' = label + blank', each step is ONE TensorE matmul with a fixed
  shift stationary (no weight reloads; targets have no adjacent repeats) plus
  VectorE adds and the emission multiply; the [blank | label+blank] pre-add
  runs concurrently with the matmul so only add+mul sit on the serial chain.
  Every R steps the state-sum (ones-column matmuls) rescales alpha; factors
  are folded back in log-space on the host.
- The emission table (101 x T*8: gathered label acts + broadcast blank col)
  is host-prepared index prep; one DMA + one exp on device.
- Final: ll_b = log(alpha_fin) + sum log u - sum log Z  (host, float64);
  loss = -mean(ll).
"""

import sys
import types

import numpy as np

# ---- shim: provide antenv.axon_hooks (missing in this image) ----------------
_HOOK = [None]
try:
    import antenv.axon_hooks  # noqa: F401
except ImportError:
    try:
        from trn_agent_boot.trn_boot import _ntff_profile_via_ctypes

        _HOOK[0] = _ntff_profile_via_ctypes("/opt/axon/libaxon_pjrt.so")
    except Exception:
        pass
    _m = types.ModuleType("antenv.axon_hooks")
    _m.get_axon_ntff_profile_hook = lambda: _HOOK[0]
    _m.set_axon_ntff_profile_hook = lambda h: _HOOK.__setitem__(0, h)
    sys.modules["antenv.axon_hooks"] = _m
# -----------------------------------------------------------------------------

import concourse.bass as bass
import concourse.mybir as mybir
import concourse.tile as tile
from concourse.bass_utils import run_bass_kernel_spmd
from concourse.vector_clock import ScopedClock


# ---- walrus-compat patches: this walrus rejects Drains with >1 sem wait -----
def _my_drain_and_barrier(self, tick_clock, wait_clock):
    nc = self.nc
    dummy = nc.sync.nop(nofuse=True)
    wait_clock.add_sem_waits(dummy.ins, ScopedClock({None: tick_clock.global_clock}))
    si = dummy.ins.sync_info
    waits = list(si.on_wait) if si is not None else []
    if si is not None and len(waits) > 1:
        dummy.ins.sync_info = mybir.SyncInfo(
            on_wait=[waits[0]], on_update=list(si.on_update)
        )
        for w in waits[1:]:
            n = nc.sync.nop(nofuse=True)
            n.ins.sync_info = mybir.SyncInfo(on_wait=[w], on_update=[])
    nc.sync.drain()
    nc.all_engine_barrier()
    assert self.sems is not None
    popped = nc._tile_sem_poison_stack.pop()
    assert popped is self._sem_poison
    nc.clear_and_free_semaphores(list(self.sems.allocated().values()))
    nc.all_engine_barrier()


def _my_multi_engine_barrier(self, engines):
    # bare per-engine drains (this walrus rejects waits on Drain) followed by
    # an EVSEM sem-only all-engine barrier for the cross-engine sync.
    for e in engines:
        self.engines[e].drain()
    for inst in self._sem_only_all_engine_barrier_insts(f"aeb{self.next_id()}"):
        self.engines[inst.engine].add_instruction(inst)


tile.TileContext._drain_and_barrier = _my_drain_and_barrier
bass.Bass.multi_engine_barrier = _my_multi_engine_barrier


def _split_multiwait(nc):
    """This walrus build encodes at most one sync-wait per instruction; hoist
    extra waits onto preceding nofuse NOPs on the same engine."""
    n_new = 0
    for fn in nc.m.functions:
        for blk in fn.blocks:
            insts = blk.instructions
            i = 0
            while i < len(insts):
                ins = insts[i]
                si = getattr(ins, "sync_info", None)
                if si is not None and si.on_wait and len(si.on_wait) > 1:
                    waits = list(si.on_wait)
                    ins.sync_info = mybir.SyncInfo(
                        on_wait=[waits[-1]], on_update=list(si.on_update)
                    )
                    new_nops = []
                    for w in waits[:-1]:
                        nop = mybir.InstNoOp(
                            name=f"{ins.name}_wsplit{n_new}",
                            engine=ins.engine,
                            sync_info=mybir.SyncInfo(on_wait=[w], on_update=[]),
                            bass_nofuse=True,
                        )
                        n_new += 1
                        new_nops.append(nop)
                    insts[i:i] = new_nops
                    i += len(new_nops)
                i += 1
    return nc
# -----------------------------------------------------------------------------

T, B, V, L = 512, 32, 8000, 100
NCORES = 8
NB = B // NCORES  # 4 samples per core
W = 2 * NB  # alpha free width: cols 0..NB-1 blank block, NB..2NB-1 label block
NBLK = L + 1  # blank states
NLAB = L  # label states
RSC = 16  # rescale every RSC steps
F32 = mybir.dt.float32
I32 = mybir.dt.int32


def n_rescales(t_steps):
    return len([t for t in range(1, t_steps) if t % RSC == 0 and t != t_steps - 1])


def build_weights():
    """Static 0/1 lhsT weight matrices [K, M] for the per-step matmuls.

    psum[:, 0:NB]   = w_b0.T @ blank + w_n0.T @ label   (new blank block)
    psum[:, NB:2NB] = w_b1.T @ blank + w_n1.T @ label   (new label block)
    blank'[j] = blank[j] + label[j-1]; label'[j] = label[j] + blank[j] + label[j-1]
    """
    w_b0 = np.zeros((NBLK, NBLK), np.float32)
    w_n0 = np.zeros((NLAB, NBLK), np.float32)
    w_b1 = np.zeros((NBLK, NBLK), np.float32)
    w_n1 = np.zeros((NLAB, NBLK), np.float32)
    for k in range(NBLK):
        w_b0[k, k] = 1.0
        if k < NLAB:
            w_b1[k, k] = 1.0
    for k in range(NLAB):
        w_n0[k, k + 1] = 1.0
        w_n1[k, k] = 1.0
        if k + 1 < NLAB:
            w_n1[k, k + 1] = 1.0
    return w_b0, w_n0, w_b1, w_n1


def build_program(t_steps=T, split=True, do_stream=True, do_rec=True):
    """Build the per-core Bass program (identical for all cores)."""
    nc = bass.Bass("TRN2", target_bir_lowering=False, debug=False)
    ntile = NB * (t_steps // 128)
    nresc = n_rescales(t_steps)

    acts_d = nc.dram_tensor("acts", [NB * t_steps, V], F32, kind="ExternalInput")
    pg_d = nc.dram_tensor("pg", [NBLK, t_steps * W], F32, kind="ExternalInput")
    w_n0_d = nc.dram_tensor("w_n0", [NLAB, NBLK], F32, kind="ExternalInput")
    e0mask_d = nc.dram_tensor("e0mask", [NBLK, W], F32, kind="ExternalInput")

    zout_d = nc.dram_tensor("zout", [ntile, 128], F32, kind="ExternalOutput")
    afin_d = nc.dram_tensor("afin", [NBLK, W], F32, kind="ExternalOutput")
    ubuf_d = nc.dram_tensor("ubuf", [1, (nresc + 1) * W], F32, kind="ExternalOutput")

    with tile.TileContext(nc) as tc:
        with (
            tc.tile_pool(name="stream", bufs=2) as stream_pool,
            tc.tile_pool(name="escratch", bufs=1) as escratch_pool,
            tc.tile_pool(name="zpool", bufs=2) as zpool,
            tc.tile_pool(name="singles", bufs=1) as singles,
            tc.tile_pool(name="alpha", bufs=6) as alpha_pool,
            tc.tile_pool(name="mainpsum", bufs=4, space="PSUM") as mainpsum,
            tc.tile_pool(name="bpsum", bufs=2, space="PSUM") as bpsum,
            tc.tile_pool(name="upsum", bufs=2, space="PSUM") as upsum,
        ):
            # ---- static small inputs -> SBUF --------------------------------
            w_n0 = singles.tile([NLAB, NBLK], F32)
            e0mask = singles.tile([NBLK, W], F32)
            ones_row = singles.tile([1, NBLK], F32)  # lhsT for bcast [1]x[101]
            ones_colk = singles.tile([NBLK, 1], F32)  # lhsT for sums [101]x[1]
            nc.sync.dma_start(out=w_n0, in_=w_n0_d[:, :])
            nc.sync.dma_start(out=e0mask, in_=e0mask_d[:, :])
            nc.vector.memset(ones_row, 1.0)
            nc.vector.memset(ones_colk, 1.0)

            # ---- emission table p~ [state 0..100, (t, col)] -----------------
            # host supplies pg = raw acts at extended labels (blank cols are
            # the blank activation broadcast across state partitions).
            phat_raw = singles.tile([NBLK, t_steps * W], F32)
            phat = singles.tile([NBLK, t_steps * W], F32)
            nc.sync.dma_start(out=phat_raw, in_=pg_d[:, :])
            nc.scalar.activation(phat, phat_raw, mybir.ActivationFunctionType.Exp)

            # ---- streaming Z = sum_v exp(acts) ------------------------------
            for it in range(ntile if do_stream else 0):
                tile_a = stream_pool.tile([128, V], F32, tag="acts")
                nc.sync.dma_start(out=tile_a, in_=acts_d[it * 128 : (it + 1) * 128, :])
                e_t = escratch_pool.tile([128, V], F32, tag="escr")
                z_t = zpool.tile([128, 1], F32, tag="z")
                nc.scalar.activation(
                    e_t, tile_a, mybir.ActivationFunctionType.Exp, accum_out=z_t
                )
                nc.sync.dma_start(out=zout_d[it : it + 1, :], in_=z_t)

            # ---- alpha recursion -------------------------------------------
            ubuf = singles.tile([1, (nresc + 1) * W], F32)
            nc.vector.memset(ubuf, 1.0)

            alpha = alpha_pool.tile([NBLK, W], F32, tag="alpha")
            nc.vector.tensor_mul(alpha, phat[:, 0:W], e0mask)

            n_resc = 0
            for t in range(1, t_steps if do_rec else 1):
                # ps = shift(label) in both col blocks; stationary w_n0 is the
                # only per-step weight -> stays resident on the PE.
                ps = mainpsum.tile([NBLK, W], F32, tag="mps")
                lab_dup = bass.AP(
                    tensor=alpha.tensor,
                    offset=alpha[0:NLAB, NB:W].offset,
                    ap=[list(alpha[0:NLAB, NB:W].ap[0]), [0, 2], [1, NB]],
                )
                nc.tensor.matmul(ps, w_n0, lab_dup, start=True, stop=True)
                # yprep = [blank | label+blank] runs concurrently with the
                # matmul; the serial tail after PE is just add + mult.
                yprep = alpha_pool.tile([NBLK, W], F32, tag="yprep")
                nc.vector.tensor_copy(yprep[:, 0:NB], alpha[:, 0:NB])
                nc.vector.tensor_add(
                    yprep[:, NB:W], alpha[:, NB:W], alpha[:, 0:NB]
                )
                y = alpha_pool.tile([NBLK, W], F32, tag="yprep")
                nc.vector.tensor_add(y, yprep, ps[0:NBLK, :])
                alpha_next = alpha_pool.tile([NBLK, W], F32, tag="alpha")
                nc.vector.tensor_mul(
                    alpha_next, y, phat[:, t * W : (t + 1) * W]
                )
                alpha = alpha_next

                if t % RSC == 0 and t != t_steps - 1:
                    # u = sum_s alpha at partition 0 via ones-column matmuls
                    pu = upsum.tile([1, W], F32, tag="ups")
                    nc.tensor.matmul(
                        pu[:, 0:NB], ones_colk, alpha[0:NBLK, 0:NB], start=True, stop=False
                    )
                    nc.tensor.matmul(
                        pu[:, 0:NB],
                        ones_colk[0:NLAB, :],
                        alpha[0:NLAB, NB:W],
                        start=False,
                        stop=True,
                    )
                    nc.vector.tensor_copy(
                        ubuf[0:1, n_resc * W : n_resc * W + NB], pu[0:1, 0:NB]
                    )
                    rrec = singles.tile([1, NB], F32, tag="rrec")
                    nc.vector.reciprocal(rrec, pu[0:1, 0:NB])
                    pb = bpsum.tile([NBLK, W], F32, tag="rbc")
                    nc.tensor.matmul(pb[:, 0:NB], ones_row, rrec, start=True, stop=True)
                    nc.tensor.matmul(pb[:, NB:W], ones_row, rrec, start=True, stop=True)
                    alpha_r = alpha_pool.tile([NBLK, W], F32, tag="alpha")
                    nc.vector.tensor_mul(alpha_r, alpha, pb)
                    alpha = alpha_r
                    n_resc += 1

            nc.sync.dma_start(out=afin_d[:, :], in_=alpha)
            nc.sync.dma_start(out=ubuf_d[:, :], in_=ubuf)
    if split:
        _split_multiwait(nc)
    return nc


_NC_CACHE = {}


def _get_program(t_steps=T):
    if t_steps not in _NC_CACHE:
        _NC_CACHE[t_steps] = build_program(t_steps)
    return _NC_CACHE[t_steps]


def make_in_maps(acts, targets, t_steps=T):
    _, w_n0, _, _ = build_weights()
    e0mask = np.zeros((NBLK, W), np.float32)
    e0mask[0, :] = 1.0
    in_maps = []
    for c in range(NCORES):
        bs = slice(c * NB, (c + 1) * NB)
        acts_c = np.ascontiguousarray(
            acts[:t_steps, bs, :].transpose(1, 0, 2).reshape(NB * t_steps, V)
        )
        tg = targets[bs]  # [NB, L]
        a = acts[:t_steps, bs, :]  # [T, NB, V]
        pg = np.zeros((NBLK, t_steps, W), np.float32)
        # label cols: pg[l, t, NB+b] = a[t, b, tg[b, l]]
        gat = a[:, np.arange(NB)[:, None], tg]  # [NB, L] adv-idx -> [T, NB, L]
        pg[0:NLAB, :, NB : NB + NB] = gat.transpose(2, 0, 1)
        # blank cols: pg[:, t, b] = a[t, b, 0] broadcast over states
        pg[:, :, 0:NB] = a[:, :, 0][None, :, :]
        pg[NLAB:, :, NB:W] = -30.0
        pg = np.ascontiguousarray(pg.reshape(NBLK, t_steps * W))
        in_maps.append(
            {
                "acts": acts_c,
                "pg": pg,
                "w_n0": w_n0,
                "e0mask": e0mask,
            }
        )
    return in_maps


def finalize(results, t_steps=T):
    """Host-side combine: per-sample log-likelihoods -> scalar loss (f64)."""
    nresc = n_rescales(t_steps)
    ntchunk = t_steps // 128
    lls = []
    for c in range(NCORES):
        out = results[c]
        zout = out["zout"].astype(np.float64)  # [ntile, 128]
        afin = out["afin"].astype(np.float64)  # [NBLK, W]
        ubuf = out["ubuf"].astype(np.float64).reshape(-1, W)  # [nresc+1, W]
        for b in range(NB):
            logz = np.log(zout[b * ntchunk : (b + 1) * ntchunk, :]).sum()
            logu = np.log(ubuf[:nresc, b]).sum() if nresc else 0.0
            fin = afin[NBLK - 1, b] + afin[NLAB - 1, NB + b]
            lls.append(np.log(fin) + logu - logz)
    return -np.sum(lls) / B


def kernel(acts, targets, act_lens, label_lens):
    acts = np.asarray(acts, np.float32)
    targets = np.asarray(targets).astype(np.int64)
    act_lens = np.asarray(act_lens)
    label_lens = np.asarray(label_lens)
    assert acts.shape == (T, B, V), acts.shape
    assert targets.shape == (B, L)
    assert (act_lens == T).all() and (label_lens == L).all(), "only full lens supported"
    assert (targets[:, 1:] != targets[:, :-1]).all(), "adjacent repeats unsupported"

    nc = _get_program(T)
    in_maps = make_in_maps(acts, targets, T)
    res = run_bass_kernel_spmd(nc, in_maps, core_ids=list(range(NCORES)))
    return np.float32(finalize(res.results, T))


if __name__ == "__main__":
    rng = np.random.default_rng(0)
    acts = rng.standard_normal((T, B, V)).astype(np.float32)
    targets = rng.integers(1, V, (B, L)).astype(np.int32)
    for bb in range(B):
        while (targets[bb, 1:] == targets[bb, :-1]).any():
            targets[bb] = rng.integers(1, V, (L,)).astype(np.int32)
    act_lens = np.full(B, T, np.int32)
    label_lens = np.full(B, L, np.int32)
    out = kernel(acts, targets, act_lens, label_lens)
    print("kernel loss:", out)
    from ctc_numpy import ctc_ref_numpy

    ref = ctc_ref_numpy(acts, targets, act_lens, label_lens)
    print("ref    loss:", ref, " rel err:", abs(out - ref) / abs(ref))



# revision 3
# speedup vs baseline: 1.5933x; 1.5933x over previous
"""CTC loss (warp-ctc semantics, size_average=True) on 8 Trainium2 NeuronCores.

Strategy (data-parallel over batch, 4 samples per core):
- Z[t,b] = sum_v exp(acts[t,b,v]) streamed as [128,8000] tiles; exp + free-dim
  sum fused in one ScalarE activation (accum_out). Host does log Z in float64.
- The alpha recursion runs in the LINEAR domain on p~ = exp(e - c_t), where
  c_t is a host-computed per-(t,sample) normalization constant chosen so the
  recursion stays centered in fp32/bf16 range with NO device rescaling (the
  constants are folded back in log-space on the host, exploiting linearity).
- Forward + backward from both ends, meeting at T/2: the CTC lattice is
  symmetric under time+state reversal (no adjacent repeats), so the backward
  suffix probabilities come from the SAME kernel structure run on a reversed
  emission table, and both chains share one stationary shift weight on the PE.
  This halves the serial chain to 255 steps per chain.
- Alpha layout [101 state-slots, 12]: cols 0-3 permanently zero, 4-7 blank
  block, 8-11 label block. Each step is 3 elementwise ops + 1 matmul:
    q = alpha[:,4:12] + alpha[:,0:8]        # = [blank | label+blank], 1 op
    ps = w_shift^T @ dup(label(alpha))      # TensorE, bf16, one stationary
    y = q + ps ; alpha' = y * phat_t        # 2 ops
  Forward chain's elementwise ops run on VectorE, backward's on GpSimd, so
  the two 255-step serial chains pipeline concurrently.
- Final: host combines alpha_mid/beta_mid across the cut in float64 with one
  transition application; ll = log P + sum c_t - sum log Z; loss = -mean(ll).
"""

import sys
import types

import numpy as np
import ml_dtypes

# ---- shim: provide antenv.axon_hooks (missing in this image) ----------------
_HOOK = [None]
try:
    import antenv.axon_hooks  # noqa: F401
except ImportError:
    try:
        from trn_agent_boot.trn_boot import _ntff_profile_via_ctypes

        _HOOK[0] = _ntff_profile_via_ctypes("/opt/axon/libaxon_pjrt.so")
    except Exception:
        pass
    _m = types.ModuleType("antenv.axon_hooks")
    _m.get_axon_ntff_profile_hook = lambda: _HOOK[0]
    _m.set_axon_ntff_profile_hook = lambda h: _HOOK.__setitem__(0, h)
    sys.modules["antenv.axon_hooks"] = _m
# -----------------------------------------------------------------------------

import concourse.bass as bass
import concourse.mybir as mybir
import concourse.tile as tile
from concourse.bass_utils import run_bass_kernel_spmd
from concourse.vector_clock import ScopedClock


# ---- walrus-compat patches: this walrus rejects Drains with >1 sem wait -----
def _my_drain_and_barrier(self, tick_clock, wait_clock):
    nc = self.nc
    dummy = nc.sync.nop(nofuse=True)
    wait_clock.add_sem_waits(dummy.ins, ScopedClock({None: tick_clock.global_clock}))
    si = dummy.ins.sync_info
    waits = list(si.on_wait) if si is not None else []
    if si is not None and len(waits) > 1:
        dummy.ins.sync_info = mybir.SyncInfo(
            on_wait=[waits[0]], on_update=list(si.on_update)
        )
        for w in waits[1:]:
            n = nc.sync.nop(nofuse=True)
            n.ins.sync_info = mybir.SyncInfo(on_wait=[w], on_update=[])
    nc.sync.drain()
    nc.all_engine_barrier()
    assert self.sems is not None
    popped = nc._tile_sem_poison_stack.pop()
    assert popped is self._sem_poison
    nc.clear_and_free_semaphores(list(self.sems.allocated().values()))
    nc.all_engine_barrier()


def _my_multi_engine_barrier(self, engines):
    for e in engines:
        self.engines[e].drain()
    for inst in self._sem_only_all_engine_barrier_insts(f"aeb{self.next_id()}"):
        self.engines[inst.engine].add_instruction(inst)


tile.TileContext._drain_and_barrier = _my_drain_and_barrier
bass.Bass.multi_engine_barrier = _my_multi_engine_barrier


def _split_multiwait(nc):
    """This walrus build encodes at most one sync-wait per instruction; hoist
    extra waits onto preceding nofuse NOPs on the same engine."""
    n_new = 0
    for fn in nc.m.functions:
        for blk in fn.blocks:
            insts = blk.instructions
            i = 0
            while i < len(insts):
                ins = insts[i]
                si = getattr(ins, "sync_info", None)
                if si is not None and si.on_wait and len(si.on_wait) > 1:
                    waits = list(si.on_wait)
                    ins.sync_info = mybir.SyncInfo(
                        on_wait=[waits[-1]], on_update=list(si.on_update)
                    )
                    new_nops = []
                    for w in waits[:-1]:
                        nop = mybir.InstNoOp(
                            name=f"{ins.name}_wsplit{n_new}",
                            engine=ins.engine,
                            sync_info=mybir.SyncInfo(on_wait=[w], on_update=[]),
                            bass_nofuse=True,
                        )
                        n_new += 1
                        new_nops.append(nop)
                    insts[i:i] = new_nops
                    i += len(new_nops)
                i += 1
    return nc
# -----------------------------------------------------------------------------

T, B, V, L = 512, 32, 8000, 100
NCORES = 8
NB = B // NCORES  # 4 samples per core
W = 2 * NB  # 8: blank block cols + label block cols
WA = 3 * NB  # 12: alpha tile width (4 zero + 4 blank + 4 label)
NBLK = L + 1  # 101 blank states / state-slots
NLAB = L  # 100 label states
TM = T // 2  # 256 emissions per chain
F32 = mybir.dt.float32
BF16 = mybir.dt.bfloat16

# host-side normalization: c_t = log(mean_s exp(e_t(s))) + CNORM
# CNORM centers the per-step log-growth of the linear-domain recursion so
# alpha stays within fp32 exponent range over 256 steps without rescaling.
CNORM = np.log(2.0) + 0.25


def build_shift_weight():
    """lhsT [NLAB, NBLK] with w[k, k+1] = 1: ps[j] = label[j-1]."""
    w = np.zeros((NLAB, NBLK), np.float32)
    for k in range(NLAB):
        w[k, k + 1] = 1.0
    return w


def build_program(t_steps=T):
    nc = bass.Bass("TRN2", target_bir_lowering=False, debug=False)
    tm = t_steps // 2
    ntile = NB * (t_steps // 128)
    ABUFS = 4

    acts_d = nc.dram_tensor("acts", [NB * t_steps, V], F32, kind="ExternalInput")
    pgf_d = nc.dram_tensor("pgf", [NBLK, tm * W], F32, kind="ExternalInput")
    pgb_d = nc.dram_tensor("pgb", [NBLK, tm * W], F32, kind="ExternalInput")
    w_n0_d = nc.dram_tensor("w_n0", [NLAB, NBLK], BF16, kind="ExternalInput")
    e0mask_d = nc.dram_tensor("e0mask", [NBLK, W], BF16, kind="ExternalInput")

    zout_d = nc.dram_tensor("zout", [ntile, 128], F32, kind="ExternalOutput")
    afinf_d = nc.dram_tensor("afinf", [NBLK, W], F32, kind="ExternalOutput")
    afinb_d = nc.dram_tensor("afinb", [NBLK, W], F32, kind="ExternalOutput")

    with tile.TileContext(nc) as tc:
        with (
            tc.tile_pool(name="stream", bufs=3) as stream_pool,
            tc.tile_pool(name="escratch", bufs=1) as escratch_pool,
            tc.tile_pool(name="zpool", bufs=2) as zpool,
            tc.tile_pool(name="singles", bufs=1) as singles,
            tc.tile_pool(name="af", bufs=ABUFS) as af_pool,
            tc.tile_pool(name="ab", bufs=ABUFS) as ab_pool,
            tc.tile_pool(name="qf", bufs=2) as qf_pool,
            tc.tile_pool(name="qb", bufs=2) as qb_pool,
            tc.tile_pool(name="psf", bufs=2, space="PSUM") as psf_pool,
            tc.tile_pool(name="psb", bufs=2, space="PSUM") as psb_pool,
        ):
            # ---- static small inputs -> SBUF --------------------------------
            w_n0 = singles.tile([NLAB, NBLK], BF16)
            e0mask = singles.tile([NBLK, W], BF16)
            nc.sync.dma_start(out=w_n0, in_=w_n0_d[:, :])
            nc.sync.dma_start(out=e0mask, in_=e0mask_d[:, :])

            pgf = singles.tile([NBLK, tm * W], F32)
            pgb = singles.tile([NBLK, tm * W], F32)
            nc.sync.dma_start(out=pgf, in_=pgf_d[:, :])
            nc.sync.dma_start(out=pgb, in_=pgb_d[:, :])

            # ---- emission tables: phat = exp(pg), bf16, in 4 chunks ---------
            phat_f = singles.tile([NBLK, tm * W], BF16)
            phat_b = singles.tile([NBLK, tm * W], BF16)
            NCHUNK = 4
            csz = tm * W // NCHUNK
            for i in range(NCHUNK):
                sl = slice(i * csz, (i + 1) * csz)
                nc.scalar.activation(
                    phat_f[:, sl], pgf[:, sl], mybir.ActivationFunctionType.Exp
                )
                nc.scalar.activation(
                    phat_b[:, sl], pgb[:, sl], mybir.ActivationFunctionType.Exp
                )

            # ---- pre-zero alpha buffers (zero cols persist across reuse) ----
            af_tiles = []
            ab_tiles = []
            for i in range(ABUFS):
                a = af_pool.tile([NBLK, WA], BF16, tag="af")
                nc.vector.memset(a, 0.0)
                af_tiles.append(a)
                b = ab_pool.tile([NBLK, WA], BF16, tag="ab")
                nc.gpsimd.memset(b, 0.0)
                ab_tiles.append(b)

            # ---- init: alpha0 = phat_0 * e0mask (states 0 and 1 only) -------
            alpha_f = af_pool.tile([NBLK, WA], BF16, tag="af")
            nc.vector.tensor_mul(alpha_f[:, NB:WA], phat_f[:, 0:W], e0mask)
            alpha_b = ab_pool.tile([NBLK, WA], BF16, tag="ab")
            nc.gpsimd.tensor_mul(alpha_b[:, NB:WA], phat_b[:, 0:W], e0mask)

            # ---- twin alpha recursions --------------------------------------
            def lab_dup(alpha):
                base = alpha[0:NLAB, 2 * NB : WA]
                return bass.AP(
                    tensor=alpha.tensor,
                    offset=base.offset,
                    ap=[list(base.ap[0]), [0, 2], [1, NB]],
                )

            for t in range(1, tm):
                tsl = slice(t * W, (t + 1) * W)
                # forward chain: PE + VectorE
                ps_f = psf_pool.tile([NBLK, W], F32, tag="psf")
                nc.tensor.matmul(ps_f, w_n0, lab_dup(alpha_f), start=True, stop=True)
                q_f = qf_pool.tile([NBLK, W], BF16, tag="qf")
                nc.vector.tensor_add(q_f, alpha_f[:, NB:WA], alpha_f[:, 0:W])
                y_f = qf_pool.tile([NBLK, W], BF16, tag="yf")
                nc.vector.tensor_add(y_f, q_f, ps_f[0:NBLK, :])
                alpha_fn = af_pool.tile([NBLK, WA], BF16, tag="af")
                nc.vector.tensor_mul(alpha_fn[:, NB:WA], y_f, phat_f[:, tsl])
                alpha_f = alpha_fn

                # backward chain: PE + GpSimd
                ps_b = psb_pool.tile([NBLK, W], F32, tag="psb")
                nc.tensor.matmul(ps_b, w_n0, lab_dup(alpha_b), start=True, stop=True)
                q_b = qb_pool.tile([NBLK, W], BF16, tag="qb")
                nc.gpsimd.tensor_add(q_b, alpha_b[:, NB:WA], alpha_b[:, 0:W])
                # GpSimd cannot read PSUM; the psum-consuming add runs on DVE
                y_b = qb_pool.tile([NBLK, W], BF16, tag="yb")
                nc.vector.tensor_add(y_b, q_b, ps_b[0:NBLK, :])
                alpha_bn = ab_pool.tile([NBLK, WA], BF16, tag="ab")
                nc.gpsimd.tensor_mul(alpha_bn[:, NB:WA], y_b, phat_b[:, tsl])
                alpha_b = alpha_bn

            # ---- final alphas -> f32 -> DRAM --------------------------------
            aff = singles.tile([NBLK, W], F32)
            nc.vector.tensor_copy(aff, alpha_f[:, NB:WA])
            nc.sync.dma_start(out=afinf_d[:, :], in_=aff)
            afb = singles.tile([NBLK, W], F32)
            nc.gpsimd.tensor_copy(afb, alpha_b[:, NB:WA])
            nc.sync.dma_start(out=afinb_d[:, :], in_=afb)

            # ---- streaming Z = sum_v exp(acts) ------------------------------
            for it in range(ntile):
                tile_a = stream_pool.tile([128, V], F32, tag="acts")
                nc.sync.dma_start(out=tile_a, in_=acts_d[it * 128 : (it + 1) * 128, :])
                e_t = escratch_pool.tile([128, V], BF16, tag="escr")
                z_t = zpool.tile([128, 1], F32, tag="z")
                nc.scalar.activation(
                    e_t, tile_a, mybir.ActivationFunctionType.Exp, accum_out=z_t
                )
                nc.sync.dma_start(out=zout_d[it : it + 1, :], in_=z_t)

    _split_multiwait(nc)
    return nc


_NC_CACHE = {}
_HOST_CACHE = {}


def _get_program(t_steps=T):
    if t_steps not in _NC_CACHE:
        _NC_CACHE[t_steps] = build_program(t_steps)
    return _NC_CACHE[t_steps]


def make_in_maps(acts, targets, t_steps=T):
    """Host prep: gathered+normalized emission tables, fwd and bwd."""
    tm = t_steps // 2
    w_n0 = build_shift_weight().astype(ml_dtypes.bfloat16)
    e0mask = np.zeros((NBLK, W), np.float32)
    e0mask[0, :] = 1.0
    e0mask = e0mask.astype(ml_dtypes.bfloat16)

    # per-(t, sample) normalization constants from gathered acts (float64)
    S = 2 * L + 1
    ext = np.zeros((B, S), np.int64)
    ext[:, 1::2] = targets
    # e_all[t, b, s] = acts[t, b, ext[b, s]]
    e_all = np.take_along_axis(
        acts.astype(np.float64), np.broadcast_to(ext[None], (t_steps, B, S)), axis=2
    )
    c_all = np.log(np.mean(np.exp(e_all), axis=2)) + CNORM  # [T, B]
    _HOST_CACHE["c_sum"] = c_all.sum(axis=0)  # [B]

    in_maps = []
    for c in range(NCORES):
        bs = slice(c * NB, (c + 1) * NB)
        acts_c = np.ascontiguousarray(
            acts[:t_steps, bs, :].transpose(1, 0, 2).reshape(NB * t_steps, V)
        )
        tg = targets[bs]  # [NB, L]
        a = acts[:t_steps, bs, :]  # [T, NB, V]
        cc = c_all[:, bs]  # [T, NB]

        # ---- forward table: t = 0..tm-1 --------------------------------
        pgf = np.full((NBLK, tm, W), -1e4, np.float32)
        gat = a[:tm, np.arange(NB)[:, None], tg]  # [tm, NB, L]
        pgf[0:NLAB, :, NB:W] = (gat - cc[:tm, :, None]).transpose(2, 0, 1)
        pgf[:, :, 0:NB] = (a[:tm, :, 0] - cc[:tm])[None, :, :]
        pgf = np.ascontiguousarray(pgf.reshape(NBLK, tm * W))

        # ---- backward table: tau = 0..tm-1 maps to t = T-1-tau,
        #      reversed label order ------------------------------------
        a_r = a[: tm - 1 : -1]  # [tm, NB, V] (t = T-1 down to tm)
        cc_r = cc[: tm - 1 : -1]  # [tm, NB]
        tg_r = tg[:, ::-1]  # reversed labels
        pgb = np.full((NBLK, tm, W), -1e4, np.float32)
        gat_r = a_r[:, np.arange(NB)[:, None], tg_r]  # [tm, NB, L]
        pgb[0:NLAB, :, NB:W] = (gat_r - cc_r[:, :, None]).transpose(2, 0, 1)
        pgb[:, :, 0:NB] = (a_r[:, :, 0] - cc_r)[None, :, :]
        pgb = np.ascontiguousarray(pgb.reshape(NBLK, tm * W))

        in_maps.append(
            {
                "acts": acts_c,
                "pgf": pgf,
                "pgb": pgb,
                "w_n0": w_n0,
                "e0mask": e0mask,
            }
        )
    return in_maps


def finalize(results, t_steps=T):
    """Host combine: meet-in-the-middle join + normalization + logZ (f64)."""
    S = 2 * L + 1
    ntchunk = t_steps // 128
    c_sum = _HOST_CACHE["c_sum"]
    lls = []
    for c in range(NCORES):
        out = results[c]
        zout = out["zout"].astype(np.float64)  # [ntile, 128]
        aff = out["afinf"].astype(np.float64)  # [NBLK, W]
        afb = out["afinb"].astype(np.float64)  # [NBLK, W]
        for b in range(NB):
            # flat alpha at t = tm-1
            al = np.zeros(S)
            al[0::2] = aff[:, b]  # blank block
            al[1::2] = aff[0:NLAB, NB + b]  # label block
            # flat beta~ (reversed coords) -> beta[s] = bt_flat[S-1-s]
            bt = np.zeros(S)
            bt[0::2] = afb[:, b]
            bt[1::2] = afb[0:NLAB, NB + b]
            beta = bt[::-1]
            # G(s) = beta[s] + beta[s+1] + (s odd)*beta[s+2]
            G = beta.copy()
            G[:-1] += beta[1:]
            G[1:-2:2] += beta[3::2]
            P = float(np.dot(al, G))
            logz = np.log(zout[b * ntchunk : (b + 1) * ntchunk, :]).sum()
            bg = c * NB + b
            lls.append(np.log(P) + c_sum[bg] - logz)
    return -np.sum(lls) / B


def kernel(acts, targets, act_lens, label_lens):
    acts = np.asarray(acts, np.float32)
    targets = np.asarray(targets).astype(np.int64)
    act_lens = np.asarray(act_lens)
    label_lens = np.asarray(label_lens)
    assert acts.shape == (T, B, V), acts.shape
    assert targets.shape == (B, L)
    assert (act_lens == T).all() and (label_lens == L).all(), "only full lens supported"
    assert (targets[:, 1:] != targets[:, :-1]).all(), "adjacent repeats unsupported"

    nc = _get_program(T)
    in_maps = make_in_maps(acts, targets, T)
    res = run_bass_kernel_spmd(nc, in_maps, core_ids=list(range(NCORES)))
    return np.float32(finalize(res.results, T))


if __name__ == "__main__":
    rng = np.random.default_rng(0)
    acts = rng.standard_normal((T, B, V)).astype(np.float32)
    targets = rng.integers(1, V, (B, L)).astype(np.int32)
    for bb in range(B):
        while (targets[bb, 1:] == targets[bb, :-1]).any():
            targets[bb] = rng.integers(1, V, (L,)).astype(np.int32)
    act_lens = np.full(B, T, np.int32)
    label_lens = np.full(B, L, np.int32)
    out = kernel(acts, targets, act_lens, label_lens)
    print("kernel loss:", out)
    from ctc_numpy import ctc_ref_numpy

    ref = ctc_ref_numpy(acts, targets, act_lens, label_lens)
    print("ref    loss:", ref, " rel err:", abs(out - ref) / abs(ref))


# revision 4
# speedup vs baseline: 3.2049x; 2.0115x over previous
"""CTC loss (warp-ctc semantics, size_average=True) on 8 Trainium2 NeuronCores.

Strategy (data-parallel over batch, 4 samples per core):
- Z[t,b] = sum_v exp(acts[t,b,v]) streamed as [128,8000] tiles; exp + free-dim
  sum fused in one ScalarE activation (accum_out) accumulating into an SBUF
  column; one tiny DMA at the end. Host does log Z in float64.
- The alpha recursion runs in the LINEAR domain on p~ = exp(e - c_t), where
  c_t is a host-computed per-(t,sample) normalization constant chosen so the
  recursion stays centered in fp32/bf16 range with NO device rescaling (the
  constants are folded back in log-space on the host, exploiting linearity).
- Forward + backward from both ends, meeting at T/2: the CTC lattice is
  symmetric under time+state reversal (no adjacent repeats), so the backward
  suffix probabilities come from the SAME recursion run on a reversed
  emission table. Both chains are FUSED into shared 16-wide tiles, so each
  time index is ONE bf16 matmul (shift, one resident stationary) plus three
  DVE ops covering both chains:
    q = alpha[:, 8:24] + alpha[:, 0:16]   # [blank|label+blank], zeros trick
    ps = w_shift^T @ dup(label cols)      # TensorE -> PSUM, both chains
    y = q + ps ; alpha' = y * phat_t      # DVE
- Alpha layout [101, 24]: cols 0-7 permanently zero, 8-15 blank (fwd 4, bwd
  4), 16-23 label (fwd 4, bwd 4).
- Final: host combines alpha_mid/beta_mid across the cut in float64 with one
  transition application; ll = log P + sum c_t - sum log Z; loss = -mean(ll).
"""

import sys
import types

import numpy as np
import ml_dtypes

# ---- shim: provide antenv.axon_hooks (missing in this image) ----------------
_HOOK = [None]
try:
    import antenv.axon_hooks  # noqa: F401
except ImportError:
    try:
        from trn_agent_boot.trn_boot import _ntff_profile_via_ctypes

        _HOOK[0] = _ntff_profile_via_ctypes("/opt/axon/libaxon_pjrt.so")
    except Exception:
        pass
    _m = types.ModuleType("antenv.axon_hooks")
    _m.get_axon_ntff_profile_hook = lambda: _HOOK[0]
    _m.set_axon_ntff_profile_hook = lambda h: _HOOK.__setitem__(0, h)
    sys.modules["antenv.axon_hooks"] = _m
# -----------------------------------------------------------------------------

import concourse.bass as bass
import concourse.mybir as mybir
import concourse.tile as tile
from concourse.bass_utils import run_bass_kernel_spmd
from concourse.vector_clock import ScopedClock


# ---- walrus-compat patches: this walrus rejects Drains with >1 sem wait -----
def _my_drain_and_barrier(self, tick_clock, wait_clock):
    nc = self.nc
    dummy = nc.sync.nop(nofuse=True)
    wait_clock.add_sem_waits(dummy.ins, ScopedClock({None: tick_clock.global_clock}))
    si = dummy.ins.sync_info
    waits = list(si.on_wait) if si is not None else []
    if si is not None and len(waits) > 1:
        dummy.ins.sync_info = mybir.SyncInfo(
            on_wait=[waits[0]], on_update=list(si.on_update)
        )
        for w in waits[1:]:
            n = nc.sync.nop(nofuse=True)
            n.ins.sync_info = mybir.SyncInfo(on_wait=[w], on_update=[])
    nc.sync.drain()
    nc.all_engine_barrier()
    assert self.sems is not None
    popped = nc._tile_sem_poison_stack.pop()
    assert popped is self._sem_poison
    nc.clear_and_free_semaphores(list(self.sems.allocated().values()))
    nc.all_engine_barrier()


def _my_multi_engine_barrier(self, engines):
    for e in engines:
        self.engines[e].drain()
    for inst in self._sem_only_all_engine_barrier_insts(f"aeb{self.next_id()}"):
        self.engines[inst.engine].add_instruction(inst)


tile.TileContext._drain_and_barrier = _my_drain_and_barrier
bass.Bass.multi_engine_barrier = _my_multi_engine_barrier


def _split_multiwait(nc):
    """This walrus build encodes at most one sync-wait per instruction; hoist
    extra waits onto preceding nofuse NOPs on the same engine."""
    n_new = 0
    for fn in nc.m.functions:
        for blk in fn.blocks:
            insts = blk.instructions
            i = 0
            while i < len(insts):
                ins = insts[i]
                si = getattr(ins, "sync_info", None)
                if si is not None and si.on_wait and len(si.on_wait) > 1:
                    waits = list(si.on_wait)
                    ins.sync_info = mybir.SyncInfo(
                        on_wait=[waits[-1]], on_update=list(si.on_update)
                    )
                    new_nops = []
                    for w in waits[:-1]:
                        nop = mybir.InstNoOp(
                            name=f"{ins.name}_wsplit{n_new}",
                            engine=ins.engine,
                            sync_info=mybir.SyncInfo(on_wait=[w], on_update=[]),
                            bass_nofuse=True,
                        )
                        n_new += 1
                        new_nops.append(nop)
                    insts[i:i] = new_nops
                    i += len(new_nops)
                i += 1
    return nc
# -----------------------------------------------------------------------------

T, B, V, L = 512, 32, 8000, 100
NCORES = 8
NB = B // NCORES  # 4 samples per core
WF = 4 * NB  # 16: fused op width (blankF blankB | labelF labelB)
WA = WF + 2 * NB  # 24: alpha tile width (8 zero + 8 blank + 8 label)
NBLK = L + 1  # 101 blank states / state-slots
NLAB = L  # 100 label states
TM = T // 2  # 256 emissions per chain
NCHUNK = 8  # pg DMA / exp chunks
F32 = mybir.dt.float32
F16 = mybir.dt.float16
BF16 = mybir.dt.bfloat16

# host-side normalization: c_t = log(mean_s exp(e_t(s))) + CNORM
CNORM = np.log(2.0) + 0.25


def build_shift_weight():
    """lhsT [NLAB, NBLK] with w[k, k+1] = 1: ps[j] = label[j-1]."""
    w = np.zeros((NLAB, NBLK), np.float32)
    for k in range(NLAB):
        w[k, k + 1] = 1.0
    return w


def build_program(t_steps=T):
    nc = bass.Bass("TRN2", target_bir_lowering=False, debug=False)
    tm = t_steps // 2
    ntile = NB * (t_steps // 128)
    ABUFS = 4

    acts_d = nc.dram_tensor("acts", [NB * t_steps, V], F32, kind="ExternalInput")
    pg_d = nc.dram_tensor("pg", [NBLK, tm * WF], F16, kind="ExternalInput")
    w_n0_d = nc.dram_tensor("w_n0", [NLAB, NBLK], BF16, kind="ExternalInput")
    e0mask_d = nc.dram_tensor("e0mask", [NBLK, WF], BF16, kind="ExternalInput")

    zout_d = nc.dram_tensor("zout", [128, ntile], F32, kind="ExternalOutput")
    afin_d = nc.dram_tensor("afin", [NBLK, WF], F32, kind="ExternalOutput")

    with tile.TileContext(nc) as tc:
        with (
            tc.tile_pool(name="stream", bufs=3) as stream_pool,
            tc.tile_pool(name="escratch", bufs=1) as escratch_pool,
            tc.tile_pool(name="singles", bufs=1) as singles,
            tc.tile_pool(name="alf", bufs=ABUFS) as al_pool,
            tc.tile_pool(name="qy", bufs=2) as qy_pool,
            tc.tile_pool(name="psp", bufs=2, space="PSUM") as ps_pool,
        ):
            # ---- static small inputs -> SBUF --------------------------------
            w_n0 = singles.tile([NLAB, NBLK], BF16)
            e0mask = singles.tile([NBLK, WF], BF16)
            nc.sync.dma_start(out=w_n0, in_=w_n0_d[:, :])
            nc.sync.dma_start(out=e0mask, in_=e0mask_d[:, :])

            # pg in NCHUNK column-chunks so the first exp chunk lands fast
            pg = singles.tile([NBLK, tm * WF], F16)
            csz = tm * WF // NCHUNK
            for i in range(NCHUNK):
                sl = slice(i * csz, (i + 1) * csz)
                nc.sync.dma_start(out=pg[:, sl], in_=pg_d[:, sl])

            # ---- emission table: phat = exp(pg), bf16, chunked --------------
            phat = singles.tile([NBLK, tm * WF], BF16)
            for i in range(NCHUNK):
                sl = slice(i * csz, (i + 1) * csz)
                nc.scalar.activation(
                    phat[:, sl], pg[:, sl], mybir.ActivationFunctionType.Exp
                )

            # ---- pre-zero alpha buffers (zero cols persist across reuse) ----
            for i in range(ABUFS):
                a = al_pool.tile([NBLK, WA], BF16, tag="al")
                nc.vector.memset(a, 0.0)

            # ---- init: alpha0 = phat_0 * e0mask (states 0 and 1 only) -------
            alpha = al_pool.tile([NBLK, WA], BF16, tag="al")
            nc.vector.tensor_mul(alpha[:, 2 * NB : WA], phat[:, 0:WF], e0mask)

            # ---- fused twin alpha recursion ---------------------------------
            LABOFF = 4 * NB  # label cols start (16)

            def lab_dup(al):
                base = al[0:NLAB, LABOFF:WA]
                return bass.AP(
                    tensor=al.tensor,
                    offset=base.offset,
                    ap=[list(base.ap[0]), [0, 2], [1, 2 * NB]],
                )

            for t in range(1, tm):
                tsl = slice(t * WF, (t + 1) * WF)
                ps = ps_pool.tile([NBLK, WF], F32, tag="ps")
                nc.tensor.matmul(ps, w_n0, lab_dup(alpha), start=True, stop=True)
                q = qy_pool.tile([NBLK, WF], BF16, tag="q")
                nc.vector.tensor_add(q, alpha[:, 2 * NB : WA], alpha[:, 0 : 4 * NB])
                y = qy_pool.tile([NBLK, WF], BF16, tag="y")
                nc.vector.tensor_add(y, q, ps[0:NBLK, :])
                alpha_n = al_pool.tile([NBLK, WA], BF16, tag="al")
                nc.vector.tensor_mul(alpha_n[:, 2 * NB : WA], y, phat[:, tsl])
                alpha = alpha_n

            # ---- final alphas -> f32 (DMA emitted after stream DMAs) --------
            aff = singles.tile([NBLK, WF], F32)
            nc.vector.tensor_copy(aff, alpha[:, 2 * NB : WA])

            # ---- streaming Z = sum_v exp(acts) ------------------------------
            ztile = singles.tile([128, ntile], F32)
            for it in range(ntile):
                tile_a = stream_pool.tile([128, V], F32, tag="acts")
                nc.sync.dma_start(out=tile_a, in_=acts_d[it * 128 : (it + 1) * 128, :])
                e_t = escratch_pool.tile([128, V], BF16, tag="escr")
                nc.scalar.activation(
                    e_t,
                    tile_a,
                    mybir.ActivationFunctionType.Exp,
                    accum_out=ztile[:, it : it + 1],
                )

            # outputs last so the SP engine never blocks the stream issue
            nc.sync.dma_start(out=afin_d[:, :], in_=aff)
            nc.sync.dma_start(out=zout_d[:, :], in_=ztile)

    _split_multiwait(nc)
    return nc


_NC_CACHE = {}
_HOST_CACHE = {}


def _get_program(t_steps=T):
    if t_steps not in _NC_CACHE:
        _NC_CACHE[t_steps] = build_program(t_steps)
    return _NC_CACHE[t_steps]


def make_in_maps(acts, targets, t_steps=T):
    """Host prep: gathered+normalized fused emission table (fwd & bwd)."""
    tm = t_steps // 2
    w_n0 = build_shift_weight().astype(ml_dtypes.bfloat16)
    e0mask = np.zeros((NBLK, WF), np.float32)
    e0mask[0, :] = 1.0
    e0mask = e0mask.astype(ml_dtypes.bfloat16)

    # per-(t, sample) normalization constants from gathered acts (float64)
    S = 2 * L + 1
    ext = np.zeros((B, S), np.int64)
    ext[:, 1::2] = targets
    e_all = np.take_along_axis(
        acts.astype(np.float64), np.broadcast_to(ext[None], (t_steps, B, S)), axis=2
    )
    c_all = np.log(np.mean(np.exp(e_all), axis=2)) + CNORM  # [T, B]
    _HOST_CACHE["c_sum"] = c_all.sum(axis=0)  # [B]

    in_maps = []
    for c in range(NCORES):
        bs = slice(c * NB, (c + 1) * NB)
        acts_c = np.ascontiguousarray(
            acts[:t_steps, bs, :].transpose(1, 0, 2).reshape(NB * t_steps, V)
        )
        tg = targets[bs]  # [NB, L]
        a = acts[:t_steps, bs, :]  # [T, NB, V]
        cc = c_all[:, bs]  # [T, NB]

        # fused table cols per t: [blankF(4) blankB(4) labelF(4) labelB(4)]
        pgt = np.full((NBLK, tm, WF), -1e4, np.float32)
        # forward: t = 0..tm-1
        gat = a[:tm, np.arange(NB)[:, None], tg]  # [tm, NB, L]
        pgt[0:NLAB, :, 2 * NB : 3 * NB] = (gat - cc[:tm, :, None]).transpose(2, 0, 1)
        pgt[:, :, 0:NB] = (a[:tm, :, 0] - cc[:tm])[None, :, :]
        # backward: tau = 0..tm-1 maps to t = T-1-tau, reversed label order
        a_r = a[: tm - 1 : -1]  # [tm, NB, V]
        cc_r = cc[: tm - 1 : -1]  # [tm, NB]
        tg_r = tg[:, ::-1]
        gat_r = a_r[:, np.arange(NB)[:, None], tg_r]  # [tm, NB, L]
        pgt[0:NLAB, :, 3 * NB : WF] = (gat_r - cc_r[:, :, None]).transpose(2, 0, 1)
        pgt[:, :, NB : 2 * NB] = (a_r[:, :, 0] - cc_r)[None, :, :]
        pgt = np.ascontiguousarray(
            pgt.reshape(NBLK, tm * WF).astype(np.float16)
        )

        in_maps.append(
            {"acts": acts_c, "pg": pgt, "w_n0": w_n0, "e0mask": e0mask}
        )
    return in_maps


def finalize(results, t_steps=T):
    """Host combine: meet-in-the-middle join + normalization + logZ (f64)."""
    S = 2 * L + 1
    ntchunk = t_steps // 128
    c_sum = _HOST_CACHE["c_sum"]
    lls = []
    for c in range(NCORES):
        out = results[c]
        zout = out["zout"].astype(np.float64)  # [128, ntile]
        afin = out["afin"].astype(np.float64)  # [NBLK, WF]
        for b in range(NB):
            # flat alpha at t = tm-1 (fwd blocks: cols b and 2NB+b)
            al = np.zeros(S)
            al[0::2] = afin[:, b]
            al[1::2] = afin[0:NLAB, 2 * NB + b]
            # flat beta~ in reversed coords (bwd blocks: cols NB+b, 3NB+b)
            bt = np.zeros(S)
            bt[0::2] = afin[:, NB + b]
            bt[1::2] = afin[0:NLAB, 3 * NB + b]
            beta = bt[::-1]
            # G(s) = beta[s] + beta[s+1] + (s odd)*beta[s+2]
            G = beta.copy()
            G[:-1] += beta[1:]
            G[1:-2:2] += beta[3::2]
            P = float(np.dot(al, G))
            logz = np.log(zout[:, b * ntchunk : (b + 1) * ntchunk]).sum()
            bg = c * NB + b
            lls.append(np.log(P) + c_sum[bg] - logz)
    return -np.sum(lls) / B


def kernel(acts, targets, act_lens, label_lens):
    acts = np.asarray(acts, np.float32)
    targets = np.asarray(targets).astype(np.int64)
    act_lens = np.asarray(act_lens)
    label_lens = np.asarray(label_lens)
    assert acts.shape == (T, B, V), acts.shape
    assert targets.shape == (B, L)
    assert (act_lens == T).all() and (label_lens == L).all(), "only full lens supported"
    assert (targets[:, 1:] != targets[:, :-1]).all(), "adjacent repeats unsupported"

    nc = _get_program(T)
    in_maps = make_in_maps(acts, targets, T)
    res = run_bass_kernel_spmd(nc, in_maps, core_ids=list(range(NCORES)))
    return np.float32(finalize(res.results, T))


if __name__ == "__main__":
    rng = np.random.default_rng(0)
    acts = rng.standard_normal((T, B, V)).astype(np.float32)
    targets = rng.integers(1, V, (B, L)).astype(np.int32)
    for bb in range(B):
        while (targets[bb, 1:] == targets[bb, :-1]).any():
            targets[bb] = rng.integers(1, V, (L,)).astype(np.int32)
    act_lens = np.full(B, T, np.int32)
    label_lens = np.full(B, L, np.int32)
    out = kernel(acts, targets, act_lens, label_lens)
    print("kernel loss:", out)
    from ctc_numpy import ctc_ref_numpy

    ref = ctc_ref_numpy(acts, targets, act_lens, label_lens)
    print("ref    loss:", ref, " rel err:", abs(out - ref) / abs(ref))
